# revision 67
# baseline (speedup 1.0000x reference)
"""Trainium2 Bass kernel for LlamaDiffSparseKVAttention.

Sharding: interleaved query-token parallel — core c owns query tokens
c, c+8, c+16, ... (256 per core) and computes ALL 32 q-heads for them:
q-projection (+RoPE), causal GQA attention over the sparsified KV cache
(all 8 KV heads resident per core), and the full output projection for
its token rows.  No collectives: the causal work per key-tile is a
contiguous suffix of each core's query set, so all 8 cores run one
identical program with perfectly balanced FLOPs.

Host precomputes the observation-window importance statistics /
quantile thresholds / top-k sparsity masks (tiny fraction of FLOPs) and
the sparsified K/V caches; the device runs everything else in fp16
operands with fp32 PSUM accumulation.
"""

import math
import numpy as np

import concourse.bass as bass
import concourse.bacc as bacc
import concourse.mybir as mybir
from concourse.tile import TileContext
from concourse.bass_utils import run_bass_kernel_spmd

B, S, HID = 1, 2048, 4096
HQ, HKV, D = 32, 8, 128
G = HQ // HKV
OBS, W, SINK = 128, 32, 2
THETA = 500000.0
TOP_FRAC, MID_SPARSITY, LOW_FRAC = 0.05, 0.7, 0.20
K_KEEP = int(math.ceil((1.0 - MID_SPARSITY) * D))
SCALE = 1.0 / math.sqrt(D)

N_CORES = 8
CORE_IDS = list(range(N_CORES))
TOK = S // N_CORES        # 256 query tokens per core
NKT = HID // 128          # 32 contraction tiles for the projections
NKEYT = S // 128          # 16 key tiles
QG = TOK // NKEYT         # 16 queries unlocked per key tile

F16 = mybir.dt.float16
F32 = mybir.dt.float32

# Pair-interleaved query columns: for head pair t, column 2*i+slot holds
# query token i of head 2t+slot.  Key tile kt covers the column suffix
# [2*QG*kt, 512) of both heads at once -> width 512-32*kt per QK matmul.
# Pack the 16 per-kt score stripes into <=512-wide PSUM bank chunks.
_CHUNKS = []
_cur, _curw = [], 0
for _kt in range(NKEYT):
    _w = 2 * (TOK - QG * _kt)
    if _curw + _w > 512:
        _CHUNKS.append(_cur)
        _cur, _curw = [], 0
    _cur.append((_kt, _curw, _w))
    _curw += _w
_CHUNKS.append(_cur)


def _rope_np(x, pos):
    half = D // 2
    inv = 1.0 / (THETA ** (np.arange(half, dtype=np.float32) / half))
    ang = pos[:, None].astype(np.float32) * inv[None, :]
    cos = np.concatenate([np.cos(ang), np.cos(ang)], -1).astype(np.float32)
    sin = np.concatenate([np.sin(ang), np.sin(ang)], -1).astype(np.float32)
    x1, x2 = x[..., :half], x[..., half:]
    rot = np.concatenate([-x2, x1], -1)
    return x * cos[None] + rot * sin[None]


def _build_program():
    nc = bacc.Bacc()

    # small, early-needed inputs declared first: input staging appears to
    # gate kernel-side DMAs per tensor in declaration order
    hs_T = nc.dram_tensor("hs_T", [128, NKT * TOK], F16, kind="ExternalInput")
    wq_a = nc.dram_tensor("wq_a", [4, 128, NKT * 128], F16, kind="ExternalInput")
    cos2 = nc.dram_tensor("cos2", [128, 2 * TOK], F32, kind="ExternalInput")
    ssin2 = nc.dram_tensor("ssin2", [128, 2 * TOK], F32, kind="ExternalInput")
    tri16 = nc.dram_tensor("tri16", [128, 2 * QG], F16, kind="ExternalInput")
    cfix2 = nc.dram_tensor("cfix2", [1, HKV * 2 * TOK], F32, kind="ExternalInput")
    ones_l = nc.dram_tensor("ones_l", [128, 1], F16, kind="ExternalInput")
    ones_r = nc.dram_tensor("ones_r", [1, 128], F16, kind="ExternalInput")
    ksp = nc.dram_tensor("ksp", [HKV, 128, S], F16, kind="ExternalInput")
    vsp = nc.dram_tensor("vsp", [HKV, 128, S], F16, kind="ExternalInput")
    wq_b = nc.dram_tensor("wq_b", [HQ - 4, 128, NKT * 128], F16, kind="ExternalInput")
    wo_d = nc.dram_tensor("wo_d", [4, HQ, 128, 1024], F16, kind="ExternalInput")
    out_ext = nc.dram_tensor("out", [TOK, HID], F16, kind="ExternalOutput")

    NPAIR = HQ // 2  # 16 head pairs; pair t = heads (2t, 2t+1), same KV head

    lp = nc.allow_low_precision(reason="fp16 operands are intentional")
    lp.__enter__()
    with TileContext(nc) as tc:
        with (
            tc.tile_pool(name="res", bufs=1) as res_pool,
            tc.tile_pool(name="oall", bufs=1) as oall_pool,
        ):
            # ---- resident tiles; DMA issue order matters: qproj pair 0 first
            hs_sb = res_pool.tile([128, NKT * TOK], F16)
            ksp_sb = res_pool.tile([128, HKV * S], F16)
            vsp_sb = res_pool.tile([128, HKV * S], F16)
            cos_sb = res_pool.tile([128, 2 * TOK], F32)
            sin_sb = res_pool.tile([128, 2 * TOK], F32)
            tri_sb = res_pool.tile([128, 2 * QG], F16)
            cfx_sb = res_pool.tile([1, HKV * 2 * TOK], F32)
            ol_sb = res_pool.tile([128, 1], F16)
            or_sb = res_pool.tile([1, 128], F16)
            o_all = oall_pool.tile([128, HQ * TOK], F16)

            def load_hs(slices):
                # hs in 4 slices so the PE can start after ~1 slice
                HSC = NKT * TOK // 4
                for h4 in slices:
                    nc.sync.dma_start(
                        out=hs_sb[:, h4 * HSC:(h4 + 1) * HSC],
                        in_=hs_T[:, h4 * HSC:(h4 + 1) * HSC],
                    )

            def load_kv(kvh):
                nc.sync.dma_start(
                    out=ksp_sb[:, kvh * S:(kvh + 1) * S], in_=ksp[kvh]
                )
                nc.sync.dma_start(
                    out=vsp_sb[:, kvh * S:(kvh + 1) * S], in_=vsp[kvh]
                )

            def load_consts():
                nc.sync.dma_start(out=cos_sb, in_=cos2[:])
                nc.sync.dma_start(out=sin_sb, in_=ssin2[:])
                nc.sync.dma_start(out=tri_sb, in_=tri16[:])
                nc.sync.dma_start(out=cfx_sb, in_=cfix2[:])
                nc.sync.dma_start(out=ol_sb, in_=ones_l[:])
                nc.sync.dma_start(out=or_sb, in_=ones_r[:])

            with (
                tc.tile_pool(name="wq", bufs=4) as wq_pool,
                tc.tile_pool(name="qt", bufs=2) as qt_pool,
                tc.tile_pool(name="y", bufs=2) as y_pool,
                tc.tile_pool(name="ek", bufs=5) as ek_pool,
                tc.tile_pool(name="ekacc", bufs=2) as ekacc_pool,
                tc.tile_pool(name="sm", bufs=2) as sm_pool,
                tc.tile_pool(name="psq", bufs=2, space="PSUM") as psq_pool,
                tc.tile_pool(name="pss", bufs=3, space="PSUM") as pss_pool,
                tc.tile_pool(name="pso", bufs=2, space="PSUM") as pso_pool,
                tc.tile_pool(name="psl", bufs=1, space="PSUM") as psl_pool,
            ):
                qts = {}
                psqs = {}
                pair_state = {}
                wq_tiles = {}

                def emit_wq_dma(t):
                    tiles = []
                    for slot in range(2):
                        wqs = wq_pool.tile(
                            [128, NKT * 128], F16, tag="wq", name=f"wq{slot}"
                        )
                        hh = 2 * t + slot
                        src = wq_a[hh] if hh < 4 else wq_b[hh - 4]
                        nc.sync.dma_start(out=wqs, in_=src)
                        tiles.append(wqs)
                    wq_tiles[t] = tiles

                def build_qp_fillers(t):
                    # KV for the upcoming head group
                    if t % 2 == 0 and t // 2 < HKV and t // 2 > 0:
                        load_kv(t // 2)
                    psq = psq_pool.tile([128, 2 * TOK], F32, tag="psq")
                    psqs[t] = psq
                    fillers = []

                    def one(slot, kt, wqs):
                        nc.tensor.matmul(
                            out=psq[:, slot * TOK:(slot + 1) * TOK],
                            lhsT=wqs[:, kt * 128:(kt + 1) * 128],
                            rhs=hs_sb[:, kt * TOK:(kt + 1) * TOK],
                            start=(kt == 0),
                            stop=(kt == NKT - 1),
                            skip_group_check=True,
                        )

                    for slot in range(2):
                        wqs = wq_tiles[t][slot]
                        for kt in range(NKT):
                            fillers.append(
                                lambda slot=slot, kt=kt, wqs=wqs: one(slot, kt, wqs)
                            )
                    del wq_tiles[t]
                    return fillers

                def emit_qp_mm(t):
                    for f in build_qp_fillers(t):
                        f()

                def rope_ops(t):
                    # rope in blocked layout, then interleave slots on the
                    # final adds via strided writes into qt; returned as
                    # closures so they can be spread across the DVE queue
                    psq = psqs.pop(t)
                    y1 = y_pool.tile([128, 2 * TOK], F32, tag="y1")
                    y2 = y_pool.tile([128, 2 * TOK], F32, tag="y2")
                    qt = qt_pool.tile([128, 2 * TOK], F16, tag="qt")
                    qts[t] = qt
                    return [
                        lambda: nc.vector.tensor_mul(y1[:], psq[:], cos_sb[:]),
                        lambda: nc.vector.tensor_mul(
                            y2[0:64, :], psq[64:128, :], sin_sb[64:128, :]),
                        lambda: nc.vector.tensor_mul(
                            y2[64:128, :], psq[0:64, :], sin_sb[0:64, :]),
                        lambda: nc.vector.tensor_add(
                            qt[:, 0:2 * TOK - 1:2], y1[:, 0:TOK], y2[:, 0:TOK]),
                        lambda: nc.vector.tensor_add(
                            qt[:, 1:2 * TOK:2], y1[:, TOK:2 * TOK], y2[:, TOK:2 * TOK]),
                    ]

                def emit_chunk_qk(t, chunk, kvh, qt):
                    cw = chunk[-1][1] + chunk[-1][2]
                    pss = pss_pool.tile([128, 512], F32, tag="pss")
                    for (kt, off, w) in chunk:
                        nc.tensor.matmul(
                            out=pss[:, off:off + w],
                            lhsT=ksp_sb[:, kvh * S + kt * 128: kvh * S + (kt + 1) * 128],
                            rhs=qt[:, 2 * QG * kt: 2 * TOK],
                            start=True,
                            stop=True,
                            skip_group_check=True,
                        )
                    ek = ek_pool.tile([128, 512], F16, tag="ek")
                    nc.scalar.activation(
                        ek[:, 0:cw], pss[:, 0:cw],
                        mybir.ActivationFunctionType.Exp, scale=SCALE,
                    )
                    for (kt, off, w) in chunk:
                        nc.vector.tensor_mul(
                            ek[:, off:off + 2 * QG], ek[:, off:off + 2 * QG], tri_sb[:]
                        )
                    return (chunk, ek)

                def emit_attn(t, fillers):
                    kvh = t // 2
                    qt = qts.pop(t)
                    ekacc = ekacc_pool.tile([128, 2 * TOK], F16, tag="ekacc")
                    pso = pso_pool.tile([128, 2 * TOK], F32, tag="pso")
                    pend = []
                    for ci in range(len(_CHUNKS)):
                        pend.append(emit_chunk_qk(t, _CHUNKS[ci], kvh, qt))
                        if ci == 1:
                            # ~2.5us of next-pair qproj on the PE while the
                            # exp pipeline fills; removes the per-pair stall
                            for _ in range(min(10, len(fillers))):
                                fillers.pop(0)()
                        if len(pend) > 3:
                            _emit_lpv(pend.pop(0), ekacc, pso, kvh)
                    for p in pend:
                        _emit_lpv(p, ekacc, pso, kvh)
                    # one 512-row ones-matmul replaces 16 region L-matmuls;
                    # the per-tile sums were folded into ekacc on the DVE
                    psl = psl_pool.tile([1, 2 * TOK], F32, tag="psl")
                    nc.tensor.matmul(
                        out=psl[:], lhsT=ol_sb[:], rhs=ekacc[:],
                        start=True, stop=True, skip_group_check=True,
                    )
                    return psl, pso

                def emit_chain(t, psl, pso):
                    # normalization chain on ACT (+one DVE sub); runs while
                    # the PE does the next pair's q-projection
                    kvh = t // 2
                    lsb = sm_pool.tile([1, 2 * TOK], F32, tag="lsb")
                    nc.scalar.copy(lsb[:], psl[:])
                    lf = sm_pool.tile([1, 2 * TOK], F32, tag="lf")
                    nc.vector.tensor_sub(
                        lf[:], lsb[:], cfx_sb[0:1, kvh * 2 * TOK:(kvh + 1) * 2 * TOK]
                    )
                    lnl = sm_pool.tile([1, 2 * TOK], F32, tag="lnl")
                    nc.scalar.activation(
                        lnl[:], lf[:], mybir.ActivationFunctionType.Ln
                    )
                    rin16 = sm_pool.tile([1, 2 * TOK], F16, tag="rin16")
                    nc.scalar.activation(
                        rin16[:], lnl[:], mybir.ActivationFunctionType.Exp, scale=-1.0
                    )
                    pair_state[t] = (pso, rin16)

                def _emit_lpv(pending, ekacc, pso, kvh):
                    chunk, ek = pending
                    for (kt, off, w) in chunk:
                        if kt == 0:
                            nc.vector.tensor_copy(ekacc[:], ek[:, off:off + w])
                        else:
                            nc.vector.tensor_add(
                                ekacc[:, 2 * QG * kt: 2 * TOK],
                                ekacc[:, 2 * QG * kt: 2 * TOK],
                                ek[:, off:off + w],
                            )
                        nc.tensor.matmul(
                            out=pso[:, 2 * QG * kt: 2 * TOK],
                            lhsT=vsp_sb[:, kvh * S + kt * 128: kvh * S + (kt + 1) * 128],
                            rhs=ek[:, off:off + w],
                            start=(kt == 0),
                            stop=(kt == NKEYT - 1),
                            skip_group_check=True,
                        )

                def emit_norm(t):
                    pso, rin16 = pair_state.pop(t)
                    # broadcast 1/l across partitions on the (otherwise idle)
                    # GpSimd engine instead of a PE matmul + ACT copy
                    rbb = sm_pool.tile([128, 2 * TOK], F16, tag="rbb")
                    nc.gpsimd.partition_broadcast(rbb[:], rin16[0:1, :])
                    nc.vector.tensor_mul(
                        o_all[:, t * 2 * TOK:(t + 1) * 2 * TOK], pso[:], rbb[:]
                    )

                # software-pipelined emission: PE always has qproj work between
                # a pair's last PV and its normalization broadcast; pair t+1's
                # rope is spread through attn(t)'s DVE queue.
                emit_wq_dma(0)
                load_hs([0, 1, 2, 3])
                load_consts()
                emit_wq_dma(1)
                emit_wq_dma(2)
                emit_qp_mm(0)
                for op in rope_ops(0):
                    op()
                emit_qp_mm(1)
                load_kv(0)
                for t in range(NPAIR):
                    if t + 3 < NPAIR:
                        emit_wq_dma(t + 3)  # prefetch ~1 pair ahead of use
                    fillers = build_qp_fillers(t + 2) if t + 2 < NPAIR else []
                    psl, pso = emit_attn(t, fillers)
                    emit_chain(t, psl, pso)
                    if t + 1 < NPAIR:
                        for op in rope_ops(t + 1):
                            op()
                    while fillers:
                        fillers.pop(0)()
                    emit_norm(t)

            # ---- output projection: out[tok, :] = o_all.T @ wo ----
            with (
                tc.tile_pool(name="wo", bufs=20) as wo_pool,
                tc.tile_pool(name="ost", bufs=4) as ost_pool,
                tc.tile_pool(name="ps2", bufs=2, space="PSUM") as ps2_pool,
            ):
                N2 = 4  # 1024-wide output column groups
                for n2 in range(N2):
                    ps = [
                        [
                            ps2_pool.tile(
                                [128, 512], F32, tag=f"ps{nh}{rt}", name=f"ps{nh}{rt}"
                            )
                            for rt in range(2)
                        ]
                        for nh in range(2)
                    ]
                    for hh in range(HQ):
                        wt = wo_pool.tile([128, 1024], F16, tag="wt")
                        eng = nc.sync if hh % 2 == 0 else nc.gpsimd
                        eng.dma_start(out=wt, in_=wo_d[n2, hh])
                        tt, slot = hh // 2, hh % 2
                        for nh in range(2):
                            for rt in range(2):
                                a0 = tt * 2 * TOK + 2 * rt * 128 + slot
                                nc.tensor.matmul(
                                    out=ps[nh][rt][:],
                                    lhsT=o_all[:, a0:a0 + 255:2],
                                    rhs=wt[:, nh * 512:(nh + 1) * 512],
                                    start=(hh == 0),
                                    stop=(hh == HQ - 1),
                                    skip_group_check=True,
                                )
                    for nh in range(2):
                        for rt in range(2):
                            ot = ost_pool.tile([128, 512], F16, tag="ot")
                            nc.vector.tensor_copy(ot[:], ps[nh][rt][:])
                            nc.sync.dma_start(
                                out=out_ext[rt * 128:(rt + 1) * 128,
                                            n2 * 1024 + nh * 512: n2 * 1024 + (nh + 1) * 512],
                                in_=ot[:],
                            )

    lp.__exit__(None, None, None)
    nc.compile()
    nc.finalize()
    return nc


_NC_CACHE = None


def _host_prep(hidden_states, wq, wk, wv):
    hs = hidden_states.reshape(S, HID).astype(np.float32)
    k = (hs @ wk).reshape(S, HKV, D).transpose(1, 0, 2)  # [8, S, D]
    v = (hs @ wv).reshape(S, HKV, D).transpose(1, 0, 2)
    k = _rope_np(k, np.arange(S)).astype(np.float32)

    obs_q = (hs[S - OBS:] @ wq).reshape(OBS, HQ, D).transpose(1, 0, 2)
    obs_q = _rope_np(obs_q, np.arange(S - OBS, S))
    obs_qg = obs_q.reshape(HKV, G, OBS, D)
    s_obs = np.einsum("hgqd,hkd->hgqk", obs_qg, k, optimize=True) * SCALE
    obs_causal = np.arange(S)[None, :] <= (S - OBS + np.arange(OBS))[:, None]
    s_obs = np.where(obs_causal[None, None], s_obs, -np.inf).astype(np.float32)
    m = s_obs.max(-1, keepdims=True)
    e = np.exp(s_obs - m)
    p = e / e.sum(-1, keepdims=True)
    aw = p.astype(np.float32).mean(1)  # [8, OBS, S]
    counts = np.minimum(OBS, S - np.arange(S)).astype(np.float32)
    imp = aw.sum(1) / counts[None, :]  # [8, S]

    imp_c = imp[:, :S - W].reshape(-1)
    t_high = np.quantile(imp_c, 1.0 - TOP_FRAC)
    t_low = np.quantile(imp_c, LOW_FRAC)
    level = np.where(imp >= t_high, 0, np.where(imp < t_low, 2, 1))
    pos = np.arange(S)
    dense = (pos >= S - W) | (pos < SINK)
    level = np.where(dense[None, :], 0, level)

    def topk_mask(x):
        a = np.abs(x)
        thr = np.sort(a, -1)[..., D - K_KEEP]
        return a >= thr[..., None]

    keep_k = np.where((level == 0)[..., None], True, (level == 1)[..., None] & topk_mask(k))
    keep_v = np.where((level == 0)[..., None], True, (level == 1)[..., None] & topk_mask(v))
    k_sp = (k * keep_k).astype(np.float32)
    v_sp = (v * keep_v).astype(np.float32)
    evicted = level == 2  # [8, S]
    cfix = np.cumsum(evicted.astype(np.float32), axis=1)  # evicted keys <= q
    return k_sp, v_sp, cfix


def kernel(hidden_states, wq, wk, wv, wo):
    global _NC_CACHE
    if _NC_CACHE is None:
        _NC_CACHE = _build_program()
    nc = _NC_CACHE

    hs = hidden_states.reshape(S, HID).astype(np.float32)
    k_sp, v_sp, cfix = _host_prep(hidden_states, wq, wk, wv)

    f16 = np.float16
    # shared across cores
    wq_pre = np.ascontiguousarray(
        wq.reshape(NKT, 128, HQ, D).transpose(2, 1, 0, 3).reshape(HQ, 128, NKT * 128)
    ).astype(f16)
    ksp_d = np.ascontiguousarray(k_sp.transpose(0, 2, 1)).astype(f16)  # [8, D, S]
    vsp_d = np.ascontiguousarray(
        v_sp.reshape(HKV, NKEYT, 128, D).transpose(0, 2, 1, 3).reshape(HKV, 128, S)
    ).astype(f16)
    # [n2, hh, p, c] tiling so each 256KB wo tile is DRAM-sequential
    wo_d = np.ascontiguousarray(
        wo.reshape(HQ, 128, 4, 1024).transpose(2, 0, 1, 3)
    ).astype(f16)
    tri_base = np.arange(128)[:, None] <= (8 * np.arange(QG))[None, :]  # c=0 base

    half = D // 2
    inv = 1.0 / (THETA ** (np.arange(half, dtype=np.float32) / half))

    in_maps = []
    for c in range(N_CORES):
        idx = c + N_CORES * np.arange(TOK)
        hs_own = hs[idx].astype(f16)  # [256, 4096]
        hs_T = np.ascontiguousarray(
            hs_own.T.reshape(NKT, 128, TOK).transpose(1, 0, 2).reshape(128, NKT * TOK)
        )
        ang = idx[:, None].astype(np.float32) * inv[None, :]  # [256, 64]
        cosb = np.cos(ang).astype(np.float32)
        sinb = np.sin(ang).astype(np.float32)
        cos1 = np.concatenate([cosb, cosb], 1).T  # [128, 256]
        ssin1 = np.concatenate([sinb, -sinb], 1).T
        # rope runs in blocked [slot0 | slot1] layout (psq is blocked)
        cos2 = np.ascontiguousarray(np.concatenate([cos1, cos1], 1))  # [128, 512]
        ssin2 = np.ascontiguousarray(np.concatenate([ssin1, ssin1], 1))
        tri1 = ((8 * np.arange(QG)[None, :] + c) >= np.arange(128)[:, None]).astype(f16)
        tri = np.repeat(tri1, 2, axis=1)  # [128, 32]
        cfo = cfix[:, idx].astype(np.float32)  # [8, 256]
        cfix2 = np.ascontiguousarray(
            np.repeat(cfo, 2, axis=1).reshape(1, HKV * 2 * TOK)
        )
        in_maps.append({
            "hs_T": hs_T,
            "wq_a": wq_pre[:4],
            "wq_b": wq_pre[4:],
            "ksp": ksp_d,
            "vsp": vsp_d,
            "cos2": cos2,
            "ssin2": ssin2,
            "tri16": np.ascontiguousarray(tri),
            "cfix2": cfix2,
            "ones_l": np.ones((128, 1), f16),
            "ones_r": np.ones((1, 128), f16),
            "wo_d": wo_d,
        })

    global LAST_RESULT
    res = run_bass_kernel_spmd(nc, in_maps, CORE_IDS, **TRACE_OPTS)
    LAST_RESULT = res
    out = np.zeros((S, HID), np.float32)
    for c in range(N_CORES):
        idx = c + N_CORES * np.arange(TOK)
        out[idx] = res.results[c]["out"].astype(np.float32)
    return out.reshape(B, S, HID)


TRACE_OPTS = {}
LAST_RESULT = None


# revision 68
# speedup vs baseline: 1.0151x; 1.0151x over previous
"""Trainium2 Bass kernel for LlamaDiffSparseKVAttention.

Sharding: interleaved query-token parallel — core c owns query tokens
c, c+8, c+16, ... (256 per core) and computes ALL 32 q-heads for them:
q-projection (+RoPE), causal GQA attention over the sparsified KV cache
(all 8 KV heads resident per core), and the full output projection for
its token rows.  No collectives: the causal work per key-tile is a
contiguous suffix of each core's query set, so all 8 cores run one
identical program with perfectly balanced FLOPs.

Host precomputes the observation-window importance statistics /
quantile thresholds / top-k sparsity masks (tiny fraction of FLOPs) and
the sparsified K/V caches; the device runs everything else in fp16
operands with fp32 PSUM accumulation.
"""

import math
import numpy as np

import concourse.bass as bass
import concourse.bacc as bacc
import concourse.mybir as mybir
from concourse.tile import TileContext
from concourse.bass_utils import run_bass_kernel_spmd

B, S, HID = 1, 2048, 4096
HQ, HKV, D = 32, 8, 128
G = HQ // HKV
OBS, W, SINK = 128, 32, 2
THETA = 500000.0
TOP_FRAC, MID_SPARSITY, LOW_FRAC = 0.05, 0.7, 0.20
K_KEEP = int(math.ceil((1.0 - MID_SPARSITY) * D))
SCALE = 1.0 / math.sqrt(D)

N_CORES = 8
CORE_IDS = list(range(N_CORES))
TOK = S // N_CORES        # 256 query tokens per core
NKT = HID // 128          # 32 contraction tiles for the projections
NKEYT = S // 128          # 16 key tiles
QG = TOK // NKEYT         # 16 queries unlocked per key tile

F16 = mybir.dt.float16
F32 = mybir.dt.float32

# Pair-interleaved query columns: for head pair t, column 2*i+slot holds
# query token i of head 2t+slot.  Key tile kt covers the column suffix
# [2*QG*kt, 512) of both heads at once -> width 512-32*kt per QK matmul.
# Pack the 16 per-kt score stripes into 512-wide PSUM bank chunks by
# pairing big+small tiles (kt=1 with 15, 2 with 14, ...): 9 uniform
# chunks instead of 11 ragged ones.  kt=0 stays first (it initializes
# pso's has_written and the ekacc copy); accumulation order of the rest
# is irrelevant.
_CHUNKS = []
for _grp in ([0], [1, 15], [2, 14], [3, 13], [4, 12], [5, 11], [6, 10], [7, 9], [8]):
    _off, _ch = 0, []
    for _kt in _grp:
        _w = 2 * (TOK - QG * _kt)
        _ch.append((_kt, _off, _w))
        _off += _w
    _CHUNKS.append(_ch)


def _rope_np(x, pos):
    half = D // 2
    inv = 1.0 / (THETA ** (np.arange(half, dtype=np.float32) / half))
    ang = pos[:, None].astype(np.float32) * inv[None, :]
    cos = np.concatenate([np.cos(ang), np.cos(ang)], -1).astype(np.float32)
    sin = np.concatenate([np.sin(ang), np.sin(ang)], -1).astype(np.float32)
    x1, x2 = x[..., :half], x[..., half:]
    rot = np.concatenate([-x2, x1], -1)
    return x * cos[None] + rot * sin[None]


def _build_program():
    nc = bacc.Bacc()

    # small, early-needed inputs declared first: input staging appears to
    # gate kernel-side DMAs per tensor in declaration order
    hs_T = nc.dram_tensor("hs_T", [128, NKT * TOK], F16, kind="ExternalInput")
    wq_a = nc.dram_tensor("wq_a", [4, 128, NKT * 128], F16, kind="ExternalInput")
    cos2 = nc.dram_tensor("cos2", [128, 2 * TOK], F32, kind="ExternalInput")
    ssin2 = nc.dram_tensor("ssin2", [128, 2 * TOK], F32, kind="ExternalInput")
    tri16 = nc.dram_tensor("tri16", [128, 2 * QG], F16, kind="ExternalInput")
    cfix2 = nc.dram_tensor("cfix2", [1, HKV * 2 * TOK], F32, kind="ExternalInput")
    ones_l = nc.dram_tensor("ones_l", [128, 1], F16, kind="ExternalInput")
    ones_r = nc.dram_tensor("ones_r", [1, 128], F16, kind="ExternalInput")
    ksp = nc.dram_tensor("ksp", [HKV, 128, S], F16, kind="ExternalInput")
    vsp = nc.dram_tensor("vsp", [HKV, 128, S], F16, kind="ExternalInput")
    wq_b = nc.dram_tensor("wq_b", [HQ - 4, 128, NKT * 128], F16, kind="ExternalInput")
    wo_d = nc.dram_tensor("wo_d", [4, HQ, 128, 1024], F16, kind="ExternalInput")
    out_ext = nc.dram_tensor("out", [TOK, HID], F16, kind="ExternalOutput")

    NPAIR = HQ // 2  # 16 head pairs; pair t = heads (2t, 2t+1), same KV head

    lp = nc.allow_low_precision(reason="fp16 operands are intentional")
    lp.__enter__()
    with TileContext(nc) as tc:
        with (
            tc.tile_pool(name="res", bufs=1) as res_pool,
            tc.tile_pool(name="oall", bufs=1) as oall_pool,
        ):
            # ---- resident tiles; DMA issue order matters: qproj pair 0 first
            hs_sb = res_pool.tile([128, NKT * TOK], F16)
            ksp_sb = res_pool.tile([128, HKV * S], F16)
            vsp_sb = res_pool.tile([128, HKV * S], F16)
            cos_sb = res_pool.tile([128, 2 * TOK], F32)
            sin_sb = res_pool.tile([128, 2 * TOK], F32)
            tri_sb = res_pool.tile([128, 2 * QG], F16)
            cfx_sb = res_pool.tile([1, HKV * 2 * TOK], F32)
            ol_sb = res_pool.tile([128, 1], F16)
            or_sb = res_pool.tile([1, 128], F16)
            o_all = oall_pool.tile([128, HQ * TOK], F16)

            def load_hs(slices):
                # hs in 4 slices so the PE can start after ~1 slice
                HSC = NKT * TOK // 4
                for h4 in slices:
                    nc.sync.dma_start(
                        out=hs_sb[:, h4 * HSC:(h4 + 1) * HSC],
                        in_=hs_T[:, h4 * HSC:(h4 + 1) * HSC],
                    )

            def load_kv(kvh):
                nc.sync.dma_start(
                    out=ksp_sb[:, kvh * S:(kvh + 1) * S], in_=ksp[kvh]
                )
                nc.sync.dma_start(
                    out=vsp_sb[:, kvh * S:(kvh + 1) * S], in_=vsp[kvh]
                )

            def load_consts():
                nc.sync.dma_start(out=cos_sb, in_=cos2[:])
                nc.sync.dma_start(out=sin_sb, in_=ssin2[:])
                nc.sync.dma_start(out=tri_sb, in_=tri16[:])
                nc.sync.dma_start(out=cfx_sb, in_=cfix2[:])
                nc.sync.dma_start(out=ol_sb, in_=ones_l[:])
                nc.sync.dma_start(out=or_sb, in_=ones_r[:])

            with (
                tc.tile_pool(name="wq", bufs=4) as wq_pool,
                tc.tile_pool(name="qt", bufs=2) as qt_pool,
                tc.tile_pool(name="y", bufs=2) as y_pool,
                tc.tile_pool(name="ek", bufs=5) as ek_pool,
                tc.tile_pool(name="ekacc", bufs=2) as ekacc_pool,
                tc.tile_pool(name="sm", bufs=2) as sm_pool,
                tc.tile_pool(name="psq", bufs=2, space="PSUM") as psq_pool,
                tc.tile_pool(name="pss", bufs=3, space="PSUM") as pss_pool,
                tc.tile_pool(name="pso", bufs=2, space="PSUM") as pso_pool,
                tc.tile_pool(name="psl", bufs=1, space="PSUM") as psl_pool,
            ):
                qts = {}
                psqs = {}
                pair_state = {}
                wq_tiles = {}

                def emit_wq_dma(t):
                    tiles = []
                    for slot in range(2):
                        wqs = wq_pool.tile(
                            [128, NKT * 128], F16, tag="wq", name=f"wq{slot}"
                        )
                        hh = 2 * t + slot
                        src = wq_a[hh] if hh < 4 else wq_b[hh - 4]
                        nc.sync.dma_start(out=wqs, in_=src)
                        tiles.append(wqs)
                    wq_tiles[t] = tiles

                def build_qp_fillers(t):
                    # KV for the upcoming head group
                    if t % 2 == 0 and t // 2 < HKV and t // 2 > 0:
                        load_kv(t // 2)
                    psq = psq_pool.tile([128, 2 * TOK], F32, tag="psq")
                    psqs[t] = psq
                    fillers = []

                    def one(slot, kt, wqs):
                        nc.tensor.matmul(
                            out=psq[:, slot * TOK:(slot + 1) * TOK],
                            lhsT=wqs[:, kt * 128:(kt + 1) * 128],
                            rhs=hs_sb[:, kt * TOK:(kt + 1) * TOK],
                            start=(kt == 0),
                            stop=(kt == NKT - 1),
                            skip_group_check=True,
                        )

                    for slot in range(2):
                        wqs = wq_tiles[t][slot]
                        for kt in range(NKT):
                            fillers.append(
                                lambda slot=slot, kt=kt, wqs=wqs: one(slot, kt, wqs)
                            )
                    del wq_tiles[t]
                    return fillers

                def emit_qp_mm(t):
                    for f in build_qp_fillers(t):
                        f()

                def rope_ops(t):
                    # rope in blocked layout, then interleave slots on the
                    # final adds via strided writes into qt; returned as
                    # closures so they can be spread across the DVE queue
                    psq = psqs.pop(t)
                    y1 = y_pool.tile([128, 2 * TOK], F32, tag="y1")
                    y2 = y_pool.tile([128, 2 * TOK], F32, tag="y2")
                    qt = qt_pool.tile([128, 2 * TOK], F16, tag="qt")
                    qts[t] = qt
                    return [
                        lambda: nc.vector.tensor_mul(y1[:], psq[:], cos_sb[:]),
                        lambda: nc.vector.tensor_mul(
                            y2[0:64, :], psq[64:128, :], sin_sb[64:128, :]),
                        lambda: nc.vector.tensor_mul(
                            y2[64:128, :], psq[0:64, :], sin_sb[0:64, :]),
                        lambda: nc.vector.tensor_add(
                            qt[:, 0:2 * TOK - 1:2], y1[:, 0:TOK], y2[:, 0:TOK]),
                        lambda: nc.vector.tensor_add(
                            qt[:, 1:2 * TOK:2], y1[:, TOK:2 * TOK], y2[:, TOK:2 * TOK]),
                    ]

                def emit_chunk_qk(t, chunk, kvh, qt):
                    cw = chunk[-1][1] + chunk[-1][2]
                    pss = pss_pool.tile([128, 512], F32, tag="pss")
                    for (kt, off, w) in chunk:
                        nc.tensor.matmul(
                            out=pss[:, off:off + w],
                            lhsT=ksp_sb[:, kvh * S + kt * 128: kvh * S + (kt + 1) * 128],
                            rhs=qt[:, 2 * QG * kt: 2 * TOK],
                            start=True,
                            stop=True,
                            skip_group_check=True,
                        )
                    ek = ek_pool.tile([128, 512], F16, tag="ek")
                    nc.scalar.activation(
                        ek[:, 0:cw], pss[:, 0:cw],
                        mybir.ActivationFunctionType.Exp, scale=SCALE,
                    )
                    for (kt, off, w) in chunk:
                        nc.vector.tensor_mul(
                            ek[:, off:off + 2 * QG], ek[:, off:off + 2 * QG], tri_sb[:]
                        )
                    return (chunk, ek)

                def emit_attn(t, fillers):
                    kvh = t // 2
                    qt = qts.pop(t)
                    ekacc = ekacc_pool.tile([128, 2 * TOK], F16, tag="ekacc")
                    pso = pso_pool.tile([128, 2 * TOK], F32, tag="pso")
                    pend = []
                    for ci in range(len(_CHUNKS)):
                        pend.append(emit_chunk_qk(t, _CHUNKS[ci], kvh, qt))
                        if ci == 1:
                            # ~2.5us of next-pair qproj on the PE while the
                            # exp pipeline fills; removes the per-pair stall
                            for _ in range(min(10, len(fillers))):
                                fillers.pop(0)()
                        if len(pend) > 3:
                            _emit_lpv(pend.pop(0), ekacc, pso, kvh)
                    for p in pend:
                        _emit_lpv(p, ekacc, pso, kvh)
                    # one 512-row ones-matmul replaces 16 region L-matmuls;
                    # the per-tile sums were folded into ekacc on the DVE
                    psl = psl_pool.tile([1, 2 * TOK], F32, tag="psl")
                    nc.tensor.matmul(
                        out=psl[:], lhsT=ol_sb[:], rhs=ekacc[:],
                        start=True, stop=True, skip_group_check=True,
                    )
                    return psl, pso

                def emit_chain(t, psl, pso):
                    # normalization chain on ACT (+one DVE sub); runs while
                    # the PE does the next pair's q-projection
                    kvh = t // 2
                    lsb = sm_pool.tile([1, 2 * TOK], F32, tag="lsb")
                    nc.scalar.copy(lsb[:], psl[:])
                    lf = sm_pool.tile([1, 2 * TOK], F32, tag="lf")
                    nc.vector.tensor_sub(
                        lf[:], lsb[:], cfx_sb[0:1, kvh * 2 * TOK:(kvh + 1) * 2 * TOK]
                    )
                    lnl = sm_pool.tile([1, 2 * TOK], F32, tag="lnl")
                    nc.scalar.activation(
                        lnl[:], lf[:], mybir.ActivationFunctionType.Ln
                    )
                    rin16 = sm_pool.tile([1, 2 * TOK], F16, tag="rin16")
                    nc.scalar.activation(
                        rin16[:], lnl[:], mybir.ActivationFunctionType.Exp, scale=-1.0
                    )
                    pair_state[t] = (pso, rin16)

                def _emit_lpv(pending, ekacc, pso, kvh):
                    chunk, ek = pending
                    for (kt, off, w) in chunk:
                        if kt == 0:
                            nc.vector.tensor_copy(ekacc[:], ek[:, off:off + w])
                        else:
                            nc.vector.tensor_add(
                                ekacc[:, 2 * QG * kt: 2 * TOK],
                                ekacc[:, 2 * QG * kt: 2 * TOK],
                                ek[:, off:off + w],
                            )
                        nc.tensor.matmul(
                            out=pso[:, 2 * QG * kt: 2 * TOK],
                            lhsT=vsp_sb[:, kvh * S + kt * 128: kvh * S + (kt + 1) * 128],
                            rhs=ek[:, off:off + w],
                            start=(kt == 0),
                            stop=(kt == NKEYT - 1),
                            skip_group_check=True,
                        )

                def emit_norm(t):
                    pso, rin16 = pair_state.pop(t)
                    # broadcast 1/l across partitions on the (otherwise idle)
                    # GpSimd engine instead of a PE matmul + ACT copy
                    rbb = sm_pool.tile([128, 2 * TOK], F16, tag="rbb")
                    nc.gpsimd.partition_broadcast(rbb[:], rin16[0:1, :])
                    nc.vector.tensor_mul(
                        o_all[:, t * 2 * TOK:(t + 1) * 2 * TOK], pso[:], rbb[:]
                    )

                # software-pipelined emission: PE always has qproj work between
                # a pair's last PV and its normalization broadcast; pair t+1's
                # rope is spread through attn(t)'s DVE queue.
                emit_wq_dma(0)
                load_hs([0, 1, 2, 3])
                load_consts()
                emit_wq_dma(1)
                emit_wq_dma(2)
                emit_qp_mm(0)
                for op in rope_ops(0):
                    op()
                emit_qp_mm(1)
                load_kv(0)
                for t in range(NPAIR):
                    if t + 3 < NPAIR:
                        emit_wq_dma(t + 3)  # prefetch ~1 pair ahead of use
                    fillers = build_qp_fillers(t + 2) if t + 2 < NPAIR else []
                    psl, pso = emit_attn(t, fillers)
                    emit_chain(t, psl, pso)
                    if t + 1 < NPAIR:
                        for op in rope_ops(t + 1):
                            op()
                    while fillers:
                        fillers.pop(0)()
                    emit_norm(t)

            # ---- output projection: out[tok, :] = o_all.T @ wo ----
            with (
                tc.tile_pool(name="wo", bufs=20) as wo_pool,
                tc.tile_pool(name="ost", bufs=4) as ost_pool,
                tc.tile_pool(name="ps2", bufs=2, space="PSUM") as ps2_pool,
            ):
                N2 = 4  # 1024-wide output column groups
                for n2 in range(N2):
                    ps = [
                        [
                            ps2_pool.tile(
                                [128, 512], F32, tag=f"ps{nh}{rt}", name=f"ps{nh}{rt}"
                            )
                            for rt in range(2)
                        ]
                        for nh in range(2)
                    ]
                    for hh in range(HQ):
                        wt = wo_pool.tile([128, 1024], F16, tag="wt")
                        eng = nc.sync if hh % 2 == 0 else nc.gpsimd
                        eng.dma_start(out=wt, in_=wo_d[n2, hh])
                        tt, slot = hh // 2, hh % 2
                        for nh in range(2):
                            for rt in range(2):
                                a0 = tt * 2 * TOK + 2 * rt * 128 + slot
                                nc.tensor.matmul(
                                    out=ps[nh][rt][:],
                                    lhsT=o_all[:, a0:a0 + 255:2],
                                    rhs=wt[:, nh * 512:(nh + 1) * 512],
                                    start=(hh == 0),
                                    stop=(hh == HQ - 1),
                                    skip_group_check=True,
                                )
                    for nh in range(2):
                        for rt in range(2):
                            ot = ost_pool.tile([128, 512], F16, tag="ot")
                            nc.vector.tensor_copy(ot[:], ps[nh][rt][:])
                            nc.sync.dma_start(
                                out=out_ext[rt * 128:(rt + 1) * 128,
                                            n2 * 1024 + nh * 512: n2 * 1024 + (nh + 1) * 512],
                                in_=ot[:],
                            )

    lp.__exit__(None, None, None)
    nc.compile()
    nc.finalize()
    return nc


_NC_CACHE = None


def _host_prep(hidden_states, wq, wk, wv):
    hs = hidden_states.reshape(S, HID).astype(np.float32)
    k = (hs @ wk).reshape(S, HKV, D).transpose(1, 0, 2)  # [8, S, D]
    v = (hs @ wv).reshape(S, HKV, D).transpose(1, 0, 2)
    k = _rope_np(k, np.arange(S)).astype(np.float32)

    obs_q = (hs[S - OBS:] @ wq).reshape(OBS, HQ, D).transpose(1, 0, 2)
    obs_q = _rope_np(obs_q, np.arange(S - OBS, S))
    obs_qg = obs_q.reshape(HKV, G, OBS, D)
    s_obs = np.einsum("hgqd,hkd->hgqk", obs_qg, k, optimize=True) * SCALE
    obs_causal = np.arange(S)[None, :] <= (S - OBS + np.arange(OBS))[:, None]
    s_obs = np.where(obs_causal[None, None], s_obs, -np.inf).astype(np.float32)
    m = s_obs.max(-1, keepdims=True)
    e = np.exp(s_obs - m)
    p = e / e.sum(-1, keepdims=True)
    aw = p.astype(np.float32).mean(1)  # [8, OBS, S]
    counts = np.minimum(OBS, S - np.arange(S)).astype(np.float32)
    imp = aw.sum(1) / counts[None, :]  # [8, S]

    imp_c = imp[:, :S - W].reshape(-1)
    t_high = np.quantile(imp_c, 1.0 - TOP_FRAC)
    t_low = np.quantile(imp_c, LOW_FRAC)
    level = np.where(imp >= t_high, 0, np.where(imp < t_low, 2, 1))
    pos = np.arange(S)
    dense = (pos >= S - W) | (pos < SINK)
    level = np.where(dense[None, :], 0, level)

    def topk_mask(x):
        a = np.abs(x)
        thr = np.sort(a, -1)[..., D - K_KEEP]
        return a >= thr[..., None]

    keep_k = np.where((level == 0)[..., None], True, (level == 1)[..., None] & topk_mask(k))
    keep_v = np.where((level == 0)[..., None], True, (level == 1)[..., None] & topk_mask(v))
    k_sp = (k * keep_k).astype(np.float32)
    v_sp = (v * keep_v).astype(np.float32)
    evicted = level == 2  # [8, S]
    cfix = np.cumsum(evicted.astype(np.float32), axis=1)  # evicted keys <= q
    return k_sp, v_sp, cfix


def kernel(hidden_states, wq, wk, wv, wo):
    global _NC_CACHE
    if _NC_CACHE is None:
        _NC_CACHE = _build_program()
    nc = _NC_CACHE

    hs = hidden_states.reshape(S, HID).astype(np.float32)
    k_sp, v_sp, cfix = _host_prep(hidden_states, wq, wk, wv)

    f16 = np.float16
    # shared across cores
    wq_pre = np.ascontiguousarray(
        wq.reshape(NKT, 128, HQ, D).transpose(2, 1, 0, 3).reshape(HQ, 128, NKT * 128)
    ).astype(f16)
    ksp_d = np.ascontiguousarray(k_sp.transpose(0, 2, 1)).astype(f16)  # [8, D, S]
    vsp_d = np.ascontiguousarray(
        v_sp.reshape(HKV, NKEYT, 128, D).transpose(0, 2, 1, 3).reshape(HKV, 128, S)
    ).astype(f16)
    # [n2, hh, p, c] tiling so each 256KB wo tile is DRAM-sequential
    wo_d = np.ascontiguousarray(
        wo.reshape(HQ, 128, 4, 1024).transpose(2, 0, 1, 3)
    ).astype(f16)
    tri_base = np.arange(128)[:, None] <= (8 * np.arange(QG))[None, :]  # c=0 base

    half = D // 2
    inv = 1.0 / (THETA ** (np.arange(half, dtype=np.float32) / half))

    in_maps = []
    for c in range(N_CORES):
        idx = c + N_CORES * np.arange(TOK)
        hs_own = hs[idx].astype(f16)  # [256, 4096]
        hs_T = np.ascontiguousarray(
            hs_own.T.reshape(NKT, 128, TOK).transpose(1, 0, 2).reshape(128, NKT * TOK)
        )
        ang = idx[:, None].astype(np.float32) * inv[None, :]  # [256, 64]
        cosb = np.cos(ang).astype(np.float32)
        sinb = np.sin(ang).astype(np.float32)
        cos1 = np.concatenate([cosb, cosb], 1).T  # [128, 256]
        ssin1 = np.concatenate([sinb, -sinb], 1).T
        # rope runs in blocked [slot0 | slot1] layout (psq is blocked)
        cos2 = np.ascontiguousarray(np.concatenate([cos1, cos1], 1))  # [128, 512]
        ssin2 = np.ascontiguousarray(np.concatenate([ssin1, ssin1], 1))
        tri1 = ((8 * np.arange(QG)[None, :] + c) >= np.arange(128)[:, None]).astype(f16)
        tri = np.repeat(tri1, 2, axis=1)  # [128, 32]
        cfo = cfix[:, idx].astype(np.float32)  # [8, 256]
        cfix2 = np.ascontiguousarray(
            np.repeat(cfo, 2, axis=1).reshape(1, HKV * 2 * TOK)
        )
        in_maps.append({
            "hs_T": hs_T,
            "wq_a": wq_pre[:4],
            "wq_b": wq_pre[4:],
            "ksp": ksp_d,
            "vsp": vsp_d,
            "cos2": cos2,
            "ssin2": ssin2,
            "tri16": np.ascontiguousarray(tri),
            "cfix2": cfix2,
            "ones_l": np.ones((128, 1), f16),
            "ones_r": np.ones((1, 128), f16),
            "wo_d": wo_d,
        })

    global LAST_RESULT
    res = run_bass_kernel_spmd(nc, in_maps, CORE_IDS, **TRACE_OPTS)
    LAST_RESULT = res
    out = np.zeros((S, HID), np.float32)
    for c in range(N_CORES):
        idx = c + N_CORES * np.arange(TOK)
        out[idx] = res.results[c]["out"].astype(np.float32)
    return out.reshape(B, S, HID)


TRACE_OPTS = {}
LAST_RESULT = None
